# revision 21
# baseline (speedup 1.0000x reference)
"""Trainium2 Bass kernel for relative-bias multi-head attention (DePOI).

Reference math (B=4, L=200, H=256, NH=4, HS=64):
    Q = queries @ Qw.T + Qb ; K = keys @ Kw.T + Kb ; V = keys @ Vw.T + Vb
    w[b,h,l,m] = qh.(kh + pKh)[m] + (tKh+dKh)[b,l,m].qh[l]   (per head, over d)
    w = w/8 ; mask ; softmax over m
    out[b,l,h,:] = sum_m a * (vh + pVh + tVh[l] + dVh[l])[m]

Distribution: 8 cores = (batch b, query-row parity p). Core c handles
b = c//2, rows l in {p, p+2, ...} (p = c%2). Row interleaving balances the
causal-truncation work (row gl only needs keys m <= gl, and the kernel only
streams/loads the first n keys of each row).

Per core: project Q/K/V once; batch-compute the shared score term
W12 = q.(K+pK) for all rows (4 matmuls) and fold the additive mask into it
(W12M -> DRAM). Rows are processed in PAIRS to halve DMA descriptor
overhead. Per row: 4 accumulating matmuls (masked-Q x tK/dK halves) + the
W12M row fetched by DMA -> exp (fused row-sum) -> recip -> aT = a.T@diag(r)
(transpose + normalize on the PE) -> V-side matmuls in float32r
(full-speed fp32 streaming) against tV | dV | (V+pV) -> per-head diagonal
blocks gathered by the final DMA.
"""

import functools
import sys

import numpy as np

for _p in ("/opt/trn_rl_repo",):
    if _p not in sys.path:
        sys.path.insert(0, _p)

import concourse.bacc as bacc
import concourse.bass as bass
import concourse.mybir as mybir
import concourse.tile as tile
from concourse.bass_utils import run_bass_kernel_spmd

B, L, H, NH, HS = 4, 200, 256, 4, 64
NPL = L // 2  # query rows per core
NPAIR = NPL // 2
NCORES = 8
FP = mybir.dt.float32
FR = mybir.dt.float32r
NEG = np.float32(-(2.0 ** 32) + 1.0)
NEG8 = np.float32(NEG * 8.0)  # pre-scale mask value (kernel scales w by 1/8)
AX = mybir.AxisListType.X
AF = mybir.ActivationFunctionType


def ctx_enter(tc, **kw):
    return tc.alloc_tile_pool(**kw)


# tuning knobs (cost-model swept); safe defaults
CFG = dict(preadd=True, psw=3, psat=3, psvo=2, tdkp=4, tdvp=3, mskp=3,
           ap=6, atp=6, vout_eng="alt", tdv_eng="sync", tdk_eng="scalar")


def _build_program(n_pair: tuple, safe: bool, repeat: int = 1):
    """Build + compile the SPMD single-core program.

    n_pair[i2] = key count for the local row pair (2*i2, 2*i2+1) — the max
    valid length over both rows and all cores; per-core / per-row differences
    are handled by the additive mask baked into W12M. `safe` keeps the
    max-subtraction in the softmax (needed only if a row is fully masked).
    """
    nc = bacc.Bacc("TRN2", target_bir_lowering=False, debug=False)

    d_tdk = nc.dram_tensor("tdk", [NPAIR, 128, L, 8], FP, kind="ExternalInput").ap()
    d_tdv = nc.dram_tensor("tdv", [NPAIR, 128, 8, H], FR, kind="ExternalInput").ap()
    d_qT = nc.dram_tensor("qT", [128, 2, NPL], FP, kind="ExternalInput").ap()
    d_kT = nc.dram_tensor("kT", [128, 2, L], FP, kind="ExternalInput").ap()
    d_pkT = nc.dram_tensor("pkT", [128, 2, L], FP, kind="ExternalInput").ap()
    d_pv = nc.dram_tensor("pv", [128, 2, H], FP, kind="ExternalInput").ap()
    d_qwT = nc.dram_tensor("qwT", [128, 2, H], FP, kind="ExternalInput").ap()
    d_kwT = nc.dram_tensor("kwT", [128, 2, H], FP, kind="ExternalInput").ap()
    d_vwT = nc.dram_tensor("vwT", [128, 2, H], FP, kind="ExternalInput").ap()
    d_qb = nc.dram_tensor("qb", [1, H], FP, kind="ExternalInput").ap()
    d_kb = nc.dram_tensor("kb", [1, H], FP, kind="ExternalInput").ap()
    d_vb = nc.dram_tensor("vb", [1, H], FP, kind="ExternalInput").ap()
    d_mrow = nc.dram_tensor("mrow", [NPL, L], FP, kind="ExternalInput").ap()
    d_eye4 = nc.dram_tensor("eye4", [NH, NH], FP, kind="ExternalInput").ap()
    d_out = nc.dram_tensor("out", [NPL, H], FP, kind="ExternalOutput").ap()
    # on-device scratch: W12M = q.(K+pK) scores + additive mask, [l, h, m]
    d_w12 = nc.dram_tensor("w12scratch", [NPL, NH, L], FP).ap()

    with tile.TileContext(nc) as tc:
        with (
            tc.tile_pool(name="consts", bufs=1) as consts,
            tc.tile_pool(name="tdkp", bufs=CFG["tdkp"]) as tdkp,
            tc.tile_pool(name="tdvp", bufs=CFG["tdvp"]) as tdvp,
            tc.tile_pool(name="mskp", bufs=CFG["mskp"]) as mskp,
            tc.tile_pool(name="ap", bufs=CFG["ap"]) as apool,
            tc.tile_pool(name="atp", bufs=CFG["atp"]) as atpool,
            tc.tile_pool(name="small", bufs=6) as small,
        ):
            psst = tc.alloc_tile_pool(name="psst", bufs=1, space="PSUM")
            # ---- load constants / small inputs ----
            qwT = consts.tile([128, 2, H], FP)
            kwT = consts.tile([128, 2, H], FP)
            vwT = consts.tile([128, 2, H], FP)
            qT = consts.tile([128, 2, NPL], FP)
            kT = consts.tile([128, 2, L], FP)
            pkT = consts.tile([128, 2, L], FP)
            pv = consts.tile([128, 2, H], FP)
            qb = consts.tile([1, H], FP)
            kb = consts.tile([1, H], FP)
            vb = consts.tile([1, H], FP)
            mrow = consts.tile([NPL, L], FP)
            eye4 = consts.tile([NH, NH], FP)
            ones = consts.tile([1, H], FP)
            for dst, src in (
                (qwT, d_qwT), (kwT, d_kwT), (vwT, d_vwT), (qT, d_qT),
                (kT, d_kT), (pkT, d_pkT), (pv, d_pv), (qb, d_qb),
                (kb, d_kb), (vb, d_vb), (mrow, d_mrow), (eye4, d_eye4),
            ):
                nc.sync.dma_start(out=dst[:], in_=src)
            nc.vector.memset(ones[:], 1.0)

            # ---- projections ----
            # QT[oh] = (queries @ Qw.T + Qb).T  half oh  -> [128, NPL]
            QT = [consts.tile([128, NPL], FP, name=f"QT{oh}") for oh in range(2)]
            for oh in range(2):
                ps = psst.tile([128, L], FP, name="setps")
                for ih in range(2):
                    nc.tensor.matmul(
                        ps[:, 0:NPL], qwT[:, ih, oh * 128:(oh + 1) * 128],
                        qT[:, ih, :], start=(ih == 0), stop=False)
                nc.tensor.matmul(ps[:, 0:NPL], qb[0:1, oh * 128:(oh + 1) * 128],
                                 ones[0:1, 0:NPL], start=False, stop=True)
                nc.vector.tensor_copy(out=QT[oh][:], in_=ps[:, 0:NPL])
            # KPT[oh] = (keys @ Kw.T + Kb).T + abs_pos_K.T  -> [128, L]
            KPT = [consts.tile([128, L], FP, name=f"KPT{oh}") for oh in range(2)]
            for oh in range(2):
                ps = psst.tile([128, L], FP, name="setps")
                for ih in range(2):
                    nc.tensor.matmul(
                        ps[:], kwT[:, ih, oh * 128:(oh + 1) * 128], kT[:, ih, :],
                        start=(ih == 0), stop=False)
                nc.tensor.matmul(ps[:], kb[0:1, oh * 128:(oh + 1) * 128],
                                 ones[0:1, 0:L], start=False, stop=True)
                nc.vector.tensor_add(out=KPT[oh][:], in0=ps[:], in1=pkT[:, oh, :])
            # VP[c] = (keys @ Vw.T + Vb + abs_pos_V) rows chunk c -> [<=128, H]
            VP = [consts.tile([128, H], FR, name=f"VP{c}") for c in range(2)]
            for c, ln in ((0, 128), (1, L - 128)):
                ps = psst.tile([128, H], FP, name="setps")
                for ih in range(2):
                    nc.tensor.matmul(
                        ps[0:ln, :], kT[:, ih, c * 128:c * 128 + ln], vwT[:, ih, :],
                        start=(ih == 0), stop=False)
                nc.tensor.matmul(ps[0:ln, :], ones[0:1, 0:ln], vb[0:1, :],
                                 start=False, stop=True)
                nc.vector.tensor_add(out=VP[c][0:ln, :], in0=ps[0:ln, :],
                                     in1=pv[0:ln, c, :])

            # ---- masked-Q blocks: QM[ih][:, 4i+h] = q[d, i] on head-h rows ----
            QM = [consts.tile([128, 4 * NPL], FP, name=f"QM{ih}") for ih in range(2)]
            for ih in range(2):
                nc.vector.memset(QM[ih][:], 0.0)
            nc.vector.tensor_copy(out=QM[0][0:64, 0::4], in_=QT[0][0:64, :])
            nc.vector.tensor_copy(out=QM[0][64:128, 1::4], in_=QT[0][64:128, :])
            nc.vector.tensor_copy(out=QM[1][0:64, 2::4], in_=QT[1][0:64, :])
            nc.vector.tensor_copy(out=QM[1][64:128, 3::4], in_=QT[1][64:128, :])

            # ---- batched W12[l,h,m] = per-head q[l].(K+pK)[m], + mask ----
            w12 = consts.tile([NPL, NH, L], FP)
            for h in range(NH):
                ps = psst.tile([128, L], FP, name="setps")
                nc.tensor.matmul(
                    ps[0:NPL, :], QT[h // 2][64 * (h % 2):64 * (h % 2) + 64, :],
                    KPT[h // 2][64 * (h % 2):64 * (h % 2) + 64, :],
                    start=True, stop=True)
                nc.vector.tensor_add(out=w12[:, h, :], in0=ps[0:NPL, :],
                                     in1=mrow[:])
            nc.sync.dma_start(out=d_w12[:, :, :], in_=w12[:])

            psst.release()
            psw = ctx_enter(tc, name="psw", bufs=CFG["psw"], space="PSUM")
            psat = ctx_enter(tc, name="psat", bufs=CFG["psat"], space="PSUM")
            psvo = ctx_enter(tc, name="psvo", bufs=CFG["psvo"], space="PSUM")

            vout4 = consts.tile([NH, H * NPL], FP)

            # ---- per query-row pair (repeat>1: timing-only work multiplier) ----
            for i2_rep in range(repeat * NPAIR):
                i2 = i2_rep % NPAIR
                n = int(n_pair[i2])
                len0 = min(n, 128)
                len1 = n - len0
                chunks = [(0, len0)] + ([(128, len1)] if len1 > 0 else [])

                tdk_t = tdkp.tile([128, L, 8], FP)
                tdk_eng = nc.sync if CFG["tdk_eng"] == "sync" else nc.scalar
                tdk_eng.dma_start(out=tdk_t[:, 0:n, :], in_=d_tdk[i2, :, 0:n, :])
                tdv_t = tdvp.tile([128, 8, H], FR)
                tdv_eng = nc.sync if CFG["tdv_eng"] == "sync" else nc.scalar
                tdv_eng.dma_start(out=tdv_t[0:len0, 0:4, :],
                                  in_=d_tdv[i2, 0:len0, 0:4, :])
                if len1 > 0:
                    tdv_eng.dma_start(out=tdv_t[0:len1, 4:8, :],
                                      in_=d_tdv[i2, 0:len1, 4:8, :])
                # fold dK into tK (j, j+2) so scores need 2 matmuls per row
                if CFG["preadd"]:
                    for par4 in (0, 4):
                        for ih in range(2):
                            nc.vector.tensor_add(
                                out=tdk_t[:, 0:n, par4 + ih],
                                in0=tdk_t[:, 0:n, par4 + ih],
                                in1=tdk_t[:, 0:n, par4 + 2 + ih])
                # W12M rows, fetched in groups of 4 pairs: [h, row, m]
                if i2 % 4 == 0:
                    g_rows = min(8, NPL - 2 * i2)
                    g_n = int(max(n_pair[i2:i2 + 4]))
                    mskq = mskp.tile([NH, 8, L], FP)
                    nc.gpsimd.dma_start(
                        out=mskq[:, 0:g_rows, 0:g_n],
                        in_=bass.AP(tensor=d_w12.tensor,
                                    offset=2 * i2 * NH * L,
                                    ap=[[L, NH], [NH * L, g_rows], [1, g_n]]))
                msk = mskq[:, 2 * (i2 % 4):2 * (i2 % 4) + 2, :]

                for par in range(2):
                    i = 2 * i2 + par
                    qm = [QM[ih][:, 4 * i:4 * i + 4] for ih in range(2)]
                    w_ps = psw.tile([NH, L], FP)
                    w = w_ps[:, 0:n]
                    nsc = 2 if CFG["preadd"] else 4
                    for jj in range(nsc):
                        nc.tensor.matmul(w, qm[jj % 2],
                                         tdk_t[:, 0:n, 4 * par + jj],
                                         start=(jj == 0), stop=(jj == nsc - 1))
                    nc.vector.tensor_add(out=w, in0=w, in1=msk[:, par, 0:n])

                    a_sc = apool.tile([NH, L], FP)
                    ssum = small.tile([NH, 1], FP, name="ssum")
                    if safe:
                        mx = small.tile([NH, 1], FP, name="mx")
                        nc.vector.reduce_max(out=mx[:], in_=w, axis=AX)
                        nm = small.tile([NH, 1], FP, name="nm")
                        nc.scalar.mul(nm[:], mx[:], -0.125)
                        nc.scalar.activation(
                            out=a_sc[:, 0:n], in_=w, func=AF.Exp,
                            bias=nm[:], scale=0.125, accum_out=ssum[:])
                    else:
                        nc.scalar.activation(
                            out=a_sc[:, 0:n], in_=w, func=AF.Exp,
                            bias=0.0, scale=0.125, accum_out=ssum[:])
                    rec = small.tile([NH, 1], FP, name="rec")
                    nc.vector.reciprocal(out=rec[:], in_=ssum[:])

                    at_sb = []
                    for st, ln in chunks:
                        at_ps = psat.tile([128, NH], FP)
                        nc.tensor.matmul(at_ps[0:ln, :], a_sc[:, st:st + ln],
                                         eye4[:], start=True, stop=True)
                        at = atpool.tile([128, NH], FR)
                        nc.vector.tensor_copy(out=at[0:ln, :], in_=at_ps[0:ln, :])
                        at_sb.append(at)

                    vo_ps = psvo.tile([NH, H], FP, name="vops")
                    n_mm = 3 * len(chunks)
                    k = 0
                    for c, (st, ln) in enumerate(chunks):
                        for rhs in (tdv_t[0:ln, 4 * c + 2 * par, :],
                                    tdv_t[0:ln, 4 * c + 2 * par + 1, :],
                                    VP[c][0:ln, :]):
                            nc.tensor.matmul(
                                vo_ps[:], at_sb[c][0:ln, :], rhs,
                                start=(k == 0), stop=(k == n_mm - 1))
                            k += 1
                    # head-diagonal blocks of vout4 row i are gathered by the
                    # final DMA; normalize by the per-head recip here
                    ve = CFG["vout_eng"]
                    if ve == "alt":
                        ve = "act" if i % 2 == 0 else "dve"
                    if ve == "act":
                        nc.scalar.mul(vout4[:, H * i:H * (i + 1)], vo_ps[:], rec[:])
                    else:
                        nc.vector.tensor_scalar_mul(
                            vout4[:, H * i:H * (i + 1)], vo_ps[:], rec[:])

            # ---- write out: out[i, 64h:64h+64] = vout4[h, 256*i + 64h ...] ----
            v4 = vout4[:].rearrange("p (l d) -> p l d", d=H)
            for h in range(NH):
                nc.sync.dma_start(
                    out=d_out[:, HS * h:HS * (h + 1)],
                    in_=v4[h:h + 1, :, HS * h:HS * (h + 1)])
            psvo.release()
            psat.release()
            psw.release()

    nc.compile()
    return nc


@functools.lru_cache(maxsize=4)
def _get_nc(n_pair: tuple, safe: bool, repeat: int = 1):
    return _build_program(n_pair, safe, repeat)


def _host_prep(inputs):
    """Build per-core input maps (sharding + layout only, no math)."""
    q = np.ascontiguousarray(inputs["queries"], np.float32)
    k = np.ascontiguousarray(inputs["keys"], np.float32)
    tK = inputs["time_matrix_K"]
    tV = inputs["time_matrix_V"]
    dK = inputs["dis_matrix_K"]
    dV = inputs["dis_matrix_V"]
    pK = np.ascontiguousarray(inputs["abs_pos_K"], np.float32)
    pV = np.ascontiguousarray(inputs["abs_pos_V"], np.float32)
    tm = np.asarray(inputs["time_mask"], bool)
    am = np.asarray(inputs["attn_mask"], bool)

    # full mask + per-row key counts (n_valid); rows fully masked keep L keys
    fm = tm[:, :, None] | am[None, :, :]  # [B, Lq, Lk]
    valid = ~fm
    any_valid = valid.any(-1)
    nv = np.where(any_valid, L - np.argmax(valid[:, :, ::-1], -1), L)  # [B, L]
    n_l = nv.reshape(B, NPL, 2).max(axis=(0, 2))  # max over cores per local row
    n_pair = tuple(int(x) for x in n_l.reshape(NPAIR, 2).max(axis=1))
    safe = bool((~any_valid).any())

    wT = {}
    for nm_, w in (("qwT", inputs["Qw"]), ("kwT", inputs["Kw"]), ("vwT", inputs["Vw"])):
        wT[nm_] = np.ascontiguousarray(
            np.asarray(w, np.float32).T.reshape(2, 128, H).transpose(1, 0, 2))
    eye4 = np.eye(NH, dtype=np.float32)

    in_maps = []
    for c in range(NCORES):
        b, p = divmod(c, 2)
        rows = np.arange(p, L, 2)
        # K-side: tdk[i2, pp, m, 4*par+j], j = (tK-lo, tK-hi, dK-lo, dK-hi)
        tdk = np.empty((NPL, 128, L, 4), np.float32)
        tKb = tK[b, rows].reshape(NPL, L, 2, 128)  # [i, m, half, pp]
        dKb = dK[b, rows].reshape(NPL, L, 2, 128)
        tdk[:, :, :, 0] = tKb[:, :, 0].transpose(0, 2, 1)
        tdk[:, :, :, 1] = tKb[:, :, 1].transpose(0, 2, 1)
        tdk[:, :, :, 2] = dKb[:, :, 0].transpose(0, 2, 1)
        tdk[:, :, :, 3] = dKb[:, :, 1].transpose(0, 2, 1)
        tdk = np.ascontiguousarray(
            tdk.reshape(NPAIR, 2, 128, L, 4).transpose(0, 2, 3, 1, 4)
        ).reshape(NPAIR, 128, L, 8)
        # V-side: tdv[i2, pp, jj, d], jj = 4c+2*par+t, c = key chunk (0:128 /
        # 128:200 rows 72:128 zero pad), t = (tV, dV)
        tdv = np.zeros((NPAIR, 2, 128, 2, 2, H), np.float32)  # [i2,par,p,c,t,d]
        tVb = tV[b, rows].reshape(NPAIR, 2, L, H)
        dVb = dV[b, rows].reshape(NPAIR, 2, L, H)
        tdv[:, :, :, 0, 0, :] = tVb[:, :, 0:128]
        tdv[:, :, :, 0, 1, :] = dVb[:, :, 0:128]
        tdv[:, :, 0:L - 128, 1, 0, :] = tVb[:, :, 128:L]
        tdv[:, :, 0:L - 128, 1, 1, :] = dVb[:, :, 128:L]
        tdv = np.ascontiguousarray(
            tdv.transpose(0, 2, 3, 1, 4, 5)).reshape(NPAIR, 128, 8, H)

        mrow = np.where(fm[b, rows], NEG8, np.float32(0.0)).astype(np.float32)
        pvb = np.zeros((128, 2, H), np.float32)
        pvb[:, 0] = pV[b, 0:128]
        pvb[0:L - 128, 1] = pV[b, 128:L]

        in_maps.append({
            "tdk": tdk,
            "tdv": tdv,
            "qT": np.ascontiguousarray(
                q[b, rows].T.reshape(2, 128, NPL).transpose(1, 0, 2)),
            "kT": np.ascontiguousarray(
                k[b].T.reshape(2, 128, L).transpose(1, 0, 2)),
            "pkT": np.ascontiguousarray(
                pK[b].T.reshape(2, 128, L).transpose(1, 0, 2)),
            "pv": pvb,
            "qwT": wT["qwT"], "kwT": wT["kwT"], "vwT": wT["vwT"],
            "qb": np.asarray(inputs["Qb"], np.float32).reshape(1, H),
            "kb": np.asarray(inputs["Kb"], np.float32).reshape(1, H),
            "vb": np.asarray(inputs["Vb"], np.float32).reshape(1, H),
            "mrow": mrow,
            "eye4": eye4,
        })
    return in_maps, n_pair, safe


PROFILE = False       # set True (e.g. from a test harness) to capture an NTFF
LAST_RESULT = None    # BassKernelResults of the most recent kernel() call


def kernel(**inputs) -> np.ndarray:
    global LAST_RESULT
    in_maps, n_pair, safe = _host_prep(inputs)
    nc = _get_nc(n_pair, safe)
    res = run_bass_kernel_spmd(
        nc, in_maps, core_ids=list(range(NCORES)), trace=PROFILE)
    LAST_RESULT = res
    out = np.empty((B, L, H), np.float32)
    for c in range(NCORES):
        b, p = divmod(c, 2)
        out[b, p::2] = res.results[c]["out"]
    return out
